# revision 1
# baseline (speedup 1.0000x reference)
# Trainium2 Bass kernel for MergedColumnParallelLinearWithTopping
# (base column-parallel GEMM + per-token LoRA "topping", Punica-style).
#
# Math per core c (of 8, column-parallel over the 2*BDIM output dim):
#   out_c = x @ Wc.T + ((x @ Ac) * Mc) @ Bc
# where Ac = concat_l A_buffer[l,:,half]  [D, L*R]
#       Bc = concat_l B_buffer[l,:,cols]  [L*R, CPC]
#       M[t, l*R+r] = (weight_indices[t] == l)   (host-precomputed one-hot)
# This turns the per-token gather into two dense GEMMs that accumulate in
# the same PSUM group as the base GEMM.  All matmuls run as float32r
# (full PE rate for moving free dim >= 256).
#
# Self-contained: hardcodes shapes, builds the Bass program, shards inputs,
# runs on cores 0-7 via run_bass_kernel_spmd, reassembles the full output.

import numpy as np

# Problem shapes (hardcoded per spec)
T, D = 2048, 2048
L, R = 16, 16
BDIM = 5632
NCORES = 8
CPC = 2 * BDIM // NCORES  # 1408 output cols per core
P = 128
KO = D // P               # 16 contraction chunks
TS = 512                  # token-slice (matmul moving free dim)
NT = T // TS              # 4
MCH = CPC // P            # 11 output-column chunks per core
LR = L * R                # 256 (one half's lora rows)
LRO = LR // P             # 2

_PROGRAM_CACHE = {}


def _build_program():
    import concourse.bacc as bacc
    import concourse.tile as tile
    from concourse import mybir

    f32 = mybir.dt.float32
    f32r = mybir.dt.float32r
    bf16 = mybir.dt.bfloat16

    nc = bacc.Bacc("TRN2", target_bir_lowering=False, debug=False)

    # All inputs arrive pre-packed on the host into SBUF layout, so every
    # DMA reads/writes long contiguous per-partition runs.
    xt_r = nc.dram_tensor("xt", [NT, P, KO, TS], f32r, kind="ExternalInput").ap()
    wt_r = nc.dram_tensor("wt", [MCH, P, KO, P], f32r, kind="ExternalInput").ap()
    ac_r = nc.dram_tensor("ac", [P, KO, LR], f32r, kind="ExternalInput").ap()
    bc_r = nc.dram_tensor("bc", [P, LRO, CPC], f32r, kind="ExternalInput").ap()
    mt_r = nc.dram_tensor("mt", [NT, P, LRO, TS], bf16, kind="ExternalInput").ap()
    out_r = nc.dram_tensor("out", [MCH, P, NT, TS], f32, kind="ExternalOutput").ap()

    with tile.TileContext(nc) as tc:
        with (
            tc.tile_pool(name="xres", bufs=NT) as xpool,
            tc.tile_pool(name="wpool", bufs=2) as wpool,
            tc.tile_pool(name="consts", bufs=1) as cpool,
            tc.tile_pool(name="mtp", bufs=NT) as mtpool,
            tc.tile_pool(name="outp", bufs=3) as outpool,
            tc.tile_pool(name="psout", bufs=4, space="PSUM") as psout,
            tc.tile_pool(name="psxa", bufs=2, space="PSUM") as psxa,
        ):
            # Split DMAs that feed matmul operands into k-groups: the fused
            # fp32r matmul's LDWEIGHTS has very few semaphore-wait slots, so
            # each matmul must depend on at most one small DMA.
            KG = 4  # k-chunks per sub-DMA

            # Constants resident in SBUF
            a_sb = cpool.tile([P, KO, LR], f32r, name="a_sb")
            for kg in range(0, KO, KG):
                nc.sync.dma_start(
                    a_sb[:, kg:kg + KG, :], ac_r[:, kg:kg + KG, :]
                )
            b_sb = cpool.tile([P, LRO, CPC], f32r, name="b_sb")
            for o in range(LRO):
                nc.sync.dma_start(b_sb[:, o, :], bc_r[:, o, :])
            # masked x@A activation, filled per token-slice below
            xam = cpool.tile([P, LRO, T], f32r, name="xam")

            # x fully resident, loaded as NT independent slices so deps are
            # per-slice (and per k-group within a slice)
            x_sb = []
            for t in range(NT):
                xs = xpool.tile([P, KO, TS], f32r, name=f"x{t}", tag="x")
                for kg in range(0, KO, KG):
                    nc.sync.dma_start(
                        xs[:, kg:kg + KG, :], xt_r[t, :, kg:kg + KG, :]
                    )
                x_sb.append(xs)

            def w_load(m):
                # single DMA per chunk: one queue, so slot-reuse WAW is one
                # semaphore and the guard absorbs the single data wait
                wtile = wpool.tile([P, KO, P], f32r, name=f"w{m}", tag="w")
                nc.sync.dma_start(wtile[:], wt_r[m])
                return wtile

            w_tiles = {0: w_load(0)}

            def base_group(m, wtile, t):
                # one [128, TS] output tile: 16 base matmuls + 2 lora matmuls
                # accumulating in the same PSUM bank
                ps = psout.tile([P, TS], f32, name=f"ps_{m}_{t}", tag="ps")
                for k in range(KO):
                    nc.tensor.matmul(
                        ps[:],
                        lhsT=wtile[:, k, :],
                        rhs=x_sb[t][:, k, :],
                        start=(k == 0),
                        stop=False,
                    )
                for k2 in range(LRO):
                    nc.tensor.matmul(
                        ps[:],
                        lhsT=b_sb[:, k2, m * P:(m + 1) * P],
                        rhs=xam[:, k2, t * TS:(t + 1) * TS],
                        start=False,
                        stop=(k2 == LRO - 1),
                    )
                o = outpool.tile([P, TS], f32, name=f"o_{m}_{t}", tag="o")
                nc.any.tensor_copy(out=o[:], in_=ps[:])
                nc.sync.dma_start(out_r[m, :, t, :], o[:])

            # Pass 1 over token-slices: compute masked XA, then first W chunk
            for t in range(NT):
                mt_sb = mtpool.tile([P, LRO, TS], bf16, name=f"mt{t}", tag="mt")
                for o in range(LRO):
                    nc.sync.dma_start(mt_sb[:, o, :], mt_r[t, :, o, :])
                for mp in range(LRO):
                    pxa = psxa.tile([P, TS], f32, name=f"pxa_{t}_{mp}", tag="pxa")
                    for k in range(KO):
                        nc.tensor.matmul(
                            pxa[:],
                            lhsT=a_sb[:, k, mp * P:(mp + 1) * P],
                            rhs=x_sb[t][:, k, :],
                            start=(k == 0),
                            stop=(k == KO - 1),
                        )
                    nc.vector.tensor_tensor(
                        xam[:, mp, t * TS:(t + 1) * TS],
                        pxa[:],
                        mt_sb[:, mp, :],
                        mybir.AluOpType.mult,
                    )
                if t == 0:
                    w_tiles[1] = w_load(1)
                    w_tiles[2] = w_load(2)
                base_group(0, w_tiles[0], t)
                base_group(1, w_tiles[1], t)

            # Remaining W chunks, x stays resident
            for m in range(2, MCH):
                if m + 1 < MCH and (m + 1) not in w_tiles:
                    w_tiles[m + 1] = w_load(m + 1)
                for t in range(NT):
                    base_group(m, w_tiles[m], t)

    nc.compile()
    return nc


def get_program():
    if "nc" not in _PROGRAM_CACHE:
        _PROGRAM_CACHE["nc"] = _build_program()
    return _PROGRAM_CACHE["nc"]


def make_in_maps(x, W, A_buffer, B_buffer, weight_indices):
    x = np.ascontiguousarray(np.asarray(x, dtype=np.float32))
    W = np.asarray(W, dtype=np.float32)
    A = np.asarray(A_buffer, dtype=np.float32)
    B = np.asarray(B_buffer, dtype=np.float32)
    wi = np.asarray(weight_indices).astype(np.int64)

    # pack to SBUF layout [.., P(partition), .., contiguous free dims]
    xt = np.ascontiguousarray(
        x.T.reshape(KO, P, NT, TS).transpose(2, 1, 0, 3)
    )  # [NT, P, KO, TS]
    onehot = (wi[None, :] == np.arange(L, dtype=wi.dtype)[:, None])
    import ml_dtypes
    mt = np.ascontiguousarray(
        np.repeat(onehot, R, axis=0)
        .reshape(LRO, P, NT, TS)
        .transpose(2, 1, 0, 3)
    ).astype(ml_dtypes.bfloat16)  # [NT, P, LRO, TS]

    in_maps = []
    for c in range(NCORES):
        h = c // 4
        lo = h * BDIM + (c % 4) * CPC
        gcols = slice(lo, lo + CPC)
        wt_c = np.ascontiguousarray(
            W[gcols, :].T.reshape(KO, P, MCH, P).transpose(2, 1, 0, 3)
        )  # [MCH, P, KO, P]
        ac_c = np.ascontiguousarray(
            A[:, :, h * R:(h + 1) * R]
            .transpose(1, 0, 2).reshape(KO, P, LR).transpose(1, 0, 2)
        )  # [P, KO, LR]
        bc_c = np.ascontiguousarray(
            B[:, :, gcols].reshape(LRO, P, CPC).transpose(1, 0, 2)
        )  # [P, LRO, CPC]
        in_maps.append({"xt": xt, "wt": wt_c, "ac": ac_c, "bc": bc_c, "mt": mt})
    return in_maps


def assemble_output(results):
    out = np.empty((T, 2 * BDIM), dtype=np.float32)
    for c in range(NCORES):
        h = c // 4
        lo = h * BDIM + (c % 4) * CPC
        # [MCH, P, NT, TS] -> [tok, col]
        piece = results[c]["out"].transpose(2, 3, 0, 1).reshape(T, CPC)
        out[:, lo:lo + CPC] = piece
    return out


def kernel(x, W, A_buffer, B_buffer, weight_indices):
    from concourse.bass_utils import run_bass_kernel_spmd

    in_maps = make_in_maps(x, W, A_buffer, B_buffer, weight_indices)
    nc = get_program()
    res = run_bass_kernel_spmd(
        nc, in_maps, core_ids=list(range(NCORES)), trace=False
    )
    return assemble_output(res.results)


def _make_runner(nc, donate=True):
    """Build a jitted 8-core runner (mirrors bass2jax.run_bass_via_pjrt).
    With donate=False, inputs/zero-outs stay device-resident across calls,
    so repeated calls re-execute the NEFF without re-uploading data."""
    import jax
    import concourse.mybir as mybir
    from jax.sharding import Mesh, NamedSharding, PartitionSpec
    from jax.experimental.shard_map import shard_map
    from concourse.bass2jax import (
        _bass_exec_p,
        install_neuronx_cc_hook,
        partition_id_tensor,
    )

    install_neuronx_cc_hook()

    partition_name = (
        nc.partition_id_tensor.name if nc.partition_id_tensor else None
    )
    in_names, out_names, out_avals, zero_outs = [], [], [], []
    for alloc in nc.m.functions[0].allocations:
        if not isinstance(alloc, mybir.MemoryLocationSet):
            continue
        name = alloc.memorylocations[0].name
        if alloc.kind == "ExternalInput":
            if name != partition_name:
                in_names.append(name)
        elif alloc.kind == "ExternalOutput":
            out_names.append(name)
            shape = tuple(alloc.tensor_shape)
            dtype = mybir.dt.np(alloc.dtype)
            out_avals.append(jax.core.ShapedArray(shape, dtype))
            zero_outs.append(np.zeros(shape, dtype))
    n_params = len(in_names)
    n_outs = len(out_avals)
    all_names = list(in_names) + list(out_names)
    if partition_name is not None:
        all_names.append(partition_name)
    all_names = tuple(all_names)

    def _body(*args):
        operands = list(args)
        if partition_name is not None:
            operands.append(partition_id_tensor())
        outs = _bass_exec_p.bind(
            *operands,
            out_avals=tuple(out_avals),
            in_names=all_names,
            out_names=tuple(out_names),
            lowering_input_output_aliases=(),
            sim_require_finite=True,
            sim_require_nnan=True,
            nc=nc,
        )
        return tuple(outs)

    devices = jax.devices()[:NCORES]
    mesh = Mesh(np.asarray(devices), ("core",))
    in_specs = (PartitionSpec("core"),) * (n_params + n_outs)
    out_specs = (PartitionSpec("core"),) * n_outs
    sharded = jax.jit(
        shard_map(
            _body, mesh=mesh, in_specs=in_specs, out_specs=out_specs,
            check_rep=False,
        ),
        donate_argnums=(
            tuple(range(n_params, n_params + n_outs)) if donate else ()
        ),
        keep_unused=True,
    )

    sharding = NamedSharding(mesh, PartitionSpec("core"))

    def put(in_maps):
        import jax
        concat_in = [
            np.concatenate([in_maps[c][name] for c in range(NCORES)], axis=0)
            for name in in_names
        ]
        concat_zeros = [
            np.zeros((NCORES * z.shape[0], *z.shape[1:]), z.dtype)
            for z in zero_outs
        ]
        return [jax.device_put(a, sharding) for a in concat_in + concat_zeros]

    def unpack(out_arrs):
        return [
            {
                name: np.asarray(out_arrs[i]).reshape(
                    NCORES, *out_avals[i].shape
                )[c]
                for i, name in enumerate(out_names)
            }
            for c in range(NCORES)
        ]

    return sharded, put, unpack


def bench(x, W, A_buffer, B_buffer, weight_indices, iters=24):
    """Returns (output, per_exec_ns, info). Fires `iters` async executions
    with device-resident inputs and blocks at the end, so per-call dispatch
    overlaps execution; the amortized delta approximates HW exec time."""
    import time

    import jax

    in_maps = make_in_maps(x, W, A_buffer, B_buffer, weight_indices)
    nc = get_program()
    sharded, put, unpack = _make_runner(nc, donate=False)
    dev_args = put(in_maps)

    outs = jax.block_until_ready(sharded(*dev_args))  # compile + warm-up
    results = unpack(outs)

    def burst(k):
        t0 = time.monotonic()
        rs = [sharded(*dev_args) for _ in range(k)]
        jax.block_until_ready(rs)
        return time.monotonic() - t0

    burst(2)  # extra warm-up
    t_small = min(burst(2) for _ in range(3))
    t_big = min(burst(2 + iters) for _ in range(3))
    per_exec_ns = (t_big - t_small) / iters * 1e9
    info = {
        "t_small_s": t_small,
        "t_big_s": t_big,
        "iters": iters,
        "per_exec_ns": per_exec_ns,
    }
    return assemble_output(results), per_exec_ns, info



# revision 5
# speedup vs baseline: 2.1272x; 2.1272x over previous
# Trainium2 Bass kernel for MergedColumnParallelLinearWithTopping
# (base column-parallel GEMM + per-token LoRA "topping", Punica-style).
#
# Math per core c (of 8, column-parallel over the 2*BDIM output dim):
#   out_c = x @ Wc.T + ((x @ Ac) * Mc) @ Bc
# with tokens SORTED by adapter id on the host (the permutation folds
# into the host-side pack and un-pack, costing nothing on device).
# Sorting makes the LoRA block-structured: adapters 0-7 ("group 0",
# 128 lora rows) own a contiguous token range, adapters 8-15 ("group
# 1") own the rest.  Per 512-token slice at most one group boundary
# crosses, so
#   - the x@A matmuls for group g only run over group g's own tokens
#     (halves the dense-mask x@A cost), and
#   - each xa@B matmul needs a single K=128 chunk instead of K=256
#     (halves the dense-mask B cost).
# The one-hot mask M (applied on DVE, off the critical PE path) still
# zeroes the 7 sibling adapters within the group.  All matmuls run as
# float32r (full PE rate for moving free dim >= 256).
#
# Self-contained: hardcodes shapes, builds the Bass program (shaped by
# the group boundary n0 derived from weight_indices at call time),
# shards inputs, runs on cores 0-7 via run_bass_kernel_spmd,
# reassembles the full output.

import numpy as np

# Problem shapes (hardcoded per spec)
T, D = 2048, 2048
L, R = 16, 16
BDIM = 5632
NCORES = 8
CPC = 2 * BDIM // NCORES  # 1408 output cols per core
P = 128
KO = D // P               # 16 contraction chunks
TS = 512                  # token-slice (matmul moving free dim)
NT = T // TS              # 4
MCH = CPC // P            # 11 output-column chunks per core
LR = L * R                # 256 (one half's lora rows)
LRO = LR // P             # 2 adapter groups of 8 (128 lora rows each)
GSZ = L // LRO            # 8 adapters per group

_PROGRAM_CACHE = {}


def _prep(weight_indices):
    """Sort tokens by adapter; n0 = #tokens in adapter group 0."""
    wi = np.asarray(weight_indices).astype(np.int64).ravel()
    perm = np.argsort(wi, kind="stable")
    wis = wi[perm]
    n0 = int(np.sum(wis < GSZ))
    return perm, wis, n0


def _slice_ranges(n0):
    """Per token-slice t: list of (group, j0, j1) sub-ranges (within-
    slice columns [j0, j1) owned by that adapter group).  fp32r
    matmuls require an even moving free dim, so the group boundary is
    rounded outward to even positions: the <=2 overlap tokens are
    computed by both groups, and the wrong group's contribution is
    exactly zero because the one-hot mask zeroes its xam rows."""
    e0 = ((n0 + 1) // 2) * 2  # group-0 end, rounded up to even
    b1 = (n0 // 2) * 2        # group-1 begin, rounded down to even
    out = []
    for t in range(NT):
        lo, hi = t * TS, (t + 1) * TS
        rs = []
        if lo < e0:
            rs.append((0, 0, min(hi, e0) - lo))
        if hi > b1:
            rs.append((1, max(lo, b1) - lo, TS))
        out.append(rs)
    return out


def _build_program(n0):
    import concourse.bacc as bacc
    import concourse.tile as tile
    from concourse import mybir

    f32 = mybir.dt.float32
    f32r = mybir.dt.float32r
    bf16 = mybir.dt.bfloat16

    ranges = _slice_ranges(n0)

    nc = bacc.Bacc("TRN2", target_bir_lowering=False, debug=False)

    # All inputs arrive pre-packed on the host into SBUF layout, so every
    # DMA reads/writes long contiguous per-partition runs.
    xt_r = nc.dram_tensor("xt", [NT, P, KO, TS], f32r, kind="ExternalInput").ap()
    wt_r = nc.dram_tensor("wt", [MCH, P, KO, P], f32r, kind="ExternalInput").ap()
    ac_r = nc.dram_tensor("ac", [P, KO, LR], f32r, kind="ExternalInput").ap()
    bc_r = nc.dram_tensor("bc", [P, LRO, CPC], f32r, kind="ExternalInput").ap()
    mt_r = nc.dram_tensor("mt", [NT, P, LRO, TS], bf16, kind="ExternalInput").ap()
    out_r = nc.dram_tensor("out", [MCH, P, NT, TS], f32, kind="ExternalOutput").ap()

    with tile.TileContext(nc) as tc:
        with (
            tc.tile_pool(name="xres", bufs=NT) as xpool,
            tc.tile_pool(name="wpool", bufs=2) as wpool,
            tc.tile_pool(name="consts", bufs=1) as cpool,
            tc.tile_pool(name="mtp", bufs=NT) as mtpool,
            tc.tile_pool(name="outp", bufs=3) as outpool,
            tc.tile_pool(name="psout", bufs=4, space="PSUM") as psout,
            tc.tile_pool(name="psxa", bufs=2, space="PSUM") as psxa,
        ):
            # Split DMAs that feed matmul operands into k-groups: the fused
            # fp32r matmul's LDWEIGHTS has very few semaphore-wait slots, so
            # each matmul must depend on at most one small DMA.
            KG = 4  # k-chunks per sub-DMA

            # Constants resident in SBUF
            a_sb = cpool.tile([P, KO, LR], f32r, name="a_sb")
            for kg in range(0, KO, KG):
                nc.sync.dma_start(
                    a_sb[:, kg:kg + KG, :], ac_r[:, kg:kg + KG, :]
                )
            b_sb = cpool.tile([P, LRO, CPC], f32r, name="b_sb")
            for o in range(LRO):
                nc.sync.dma_start(b_sb[:, o, :], bc_r[:, o, :])
            # masked x@A activation, filled per (group, token sub-range)
            xam = cpool.tile([P, LRO, T], f32r, name="xam")

            # x fully resident, loaded as NT independent slices so deps are
            # per-slice (and per k-group within a slice)
            x_sb = []
            for t in range(NT):
                xs = xpool.tile([P, KO, TS], f32r, name=f"x{t}", tag="x")
                for kg in range(0, KO, KG):
                    nc.sync.dma_start(
                        xs[:, kg:kg + KG, :], xt_r[t, :, kg:kg + KG, :]
                    )
                x_sb.append(xs)

            def w_load(m):
                # single DMA per chunk: one queue, so slot-reuse WAW is one
                # semaphore and the guard absorbs the single data wait
                wtile = wpool.tile([P, KO, P], f32r, name=f"w{m}", tag="w")
                nc.sync.dma_start(wtile[:], wt_r[m])
                return wtile

            w_tiles = {0: w_load(0)}

            def base_group(m, wtile, t):
                # one [128, TS] output tile: 16 base matmuls + the lora B
                # matmuls for the groups present in this slice, all
                # accumulating in the same PSUM bank.  The last base
                # matmul closes the group over the full free range.
                ps = psout.tile([P, TS], f32, name=f"ps_{m}_{t}", tag="ps")
                for k in range(KO - 1):
                    nc.tensor.matmul(
                        ps[:],
                        lhsT=wtile[:, k, :],
                        rhs=x_sb[t][:, k, :],
                        start=(k == 0),
                        stop=False,
                    )
                for (g, j0, j1) in ranges[t]:
                    nc.tensor.matmul(
                        ps[:, j0:j1],
                        lhsT=b_sb[:, g, m * P:(m + 1) * P],
                        rhs=xam[:, g, t * TS + j0:t * TS + j1],
                        start=False,
                        stop=False,
                    )
                k = KO - 1
                nc.tensor.matmul(
                    ps[:],
                    lhsT=wtile[:, k, :],
                    rhs=x_sb[t][:, k, :],
                    start=False,
                    stop=True,
                )
                o = outpool.tile([P, TS], f32, name=f"o_{m}_{t}", tag="o")
                nc.any.tensor_copy(out=o[:], in_=ps[:])
                nc.sync.dma_start(out_r[m, :, t, :], o[:])

            # Pass 1 over token-slices: compute masked XA for the groups
            # present in each slice, then the first W chunks
            for t in range(NT):
                mt_sb = mtpool.tile([P, LRO, TS], bf16, name=f"mt{t}", tag="mt")
                for (g, j0, j1) in ranges[t]:
                    nc.sync.dma_start(mt_sb[:, g, j0:j1], mt_r[t, :, g, j0:j1])
                for (g, j0, j1) in ranges[t]:
                    # full-bank PSUM tile (bank-aligned) sliced to the
                    # sub-range; odd-sized PSUM allocations can land at
                    # non-bank-aligned offsets the fp32r matmul rejects
                    pxa = psxa.tile([P, TS], f32, name=f"pxa_{t}_{g}",
                                    tag="pxa")
                    for k in range(KO):
                        nc.tensor.matmul(
                            pxa[:, 0:j1 - j0],
                            lhsT=a_sb[:, k, g * P:(g + 1) * P],
                            rhs=x_sb[t][:, k, j0:j1],
                            start=(k == 0),
                            stop=(k == KO - 1),
                        )
                    nc.vector.tensor_tensor(
                        xam[:, g, t * TS + j0:t * TS + j1],
                        pxa[:, 0:j1 - j0],
                        mt_sb[:, g, j0:j1],
                        mybir.AluOpType.mult,
                    )
                if t == 0:
                    w_tiles[1] = w_load(1)
                    w_tiles[2] = w_load(2)
                base_group(0, w_tiles[0], t)
                base_group(1, w_tiles[1], t)

            # Remaining W chunks, x stays resident
            for m in range(2, MCH):
                if m + 1 < MCH and (m + 1) not in w_tiles:
                    w_tiles[m + 1] = w_load(m + 1)
                for t in range(NT):
                    base_group(m, w_tiles[m], t)

    nc.compile()
    return nc


def get_program(n0):
    if n0 not in _PROGRAM_CACHE:
        _PROGRAM_CACHE[n0] = _build_program(n0)
    return _PROGRAM_CACHE[n0]


def make_in_maps(x, W, A_buffer, B_buffer, weight_indices, perm=None,
                 wis=None):
    x = np.ascontiguousarray(np.asarray(x, dtype=np.float32))
    W = np.asarray(W, dtype=np.float32)
    A = np.asarray(A_buffer, dtype=np.float32)
    B = np.asarray(B_buffer, dtype=np.float32)
    if perm is None:
        perm, wis, _ = _prep(weight_indices)

    xs_srt = x[perm]
    # pack to SBUF layout [.., P(partition), .., contiguous free dims]
    xt = np.ascontiguousarray(
        xs_srt.T.reshape(KO, P, NT, TS).transpose(2, 1, 0, 3)
    )  # [NT, P, KO, TS]
    onehot = (wis[None, :] == np.arange(L, dtype=wis.dtype)[:, None])
    import ml_dtypes
    mt = np.ascontiguousarray(
        np.repeat(onehot, R, axis=0)
        .reshape(LRO, P, NT, TS)
        .transpose(2, 1, 0, 3)
    ).astype(ml_dtypes.bfloat16)  # [NT, P, LRO, TS]

    in_maps = []
    for c in range(NCORES):
        h = c // 4
        lo = h * BDIM + (c % 4) * CPC
        gcols = slice(lo, lo + CPC)
        wt_c = np.ascontiguousarray(
            W[gcols, :].T.reshape(KO, P, MCH, P).transpose(2, 1, 0, 3)
        )  # [MCH, P, KO, P]
        ac_c = np.ascontiguousarray(
            A[:, :, h * R:(h + 1) * R]
            .transpose(1, 0, 2).reshape(KO, P, LR).transpose(1, 0, 2)
        )  # [P, KO, LR]
        bc_c = np.ascontiguousarray(
            B[:, :, gcols].reshape(LRO, P, CPC).transpose(1, 0, 2)
        )  # [P, LRO, CPC]
        in_maps.append({"xt": xt, "wt": wt_c, "ac": ac_c, "bc": bc_c, "mt": mt})
    return in_maps


def assemble_output(results, perm):
    out = np.empty((T, 2 * BDIM), dtype=np.float32)
    for c in range(NCORES):
        h = c // 4
        lo = h * BDIM + (c % 4) * CPC
        # [MCH, P, NT, TS] -> [sorted tok, col]
        piece = results[c]["out"].transpose(2, 3, 0, 1).reshape(T, CPC)
        out[perm, lo:lo + CPC] = piece
    return out


def kernel(x, W, A_buffer, B_buffer, weight_indices):
    from concourse.bass_utils import run_bass_kernel_spmd

    perm, wis, n0 = _prep(weight_indices)
    in_maps = make_in_maps(x, W, A_buffer, B_buffer, weight_indices,
                           perm=perm, wis=wis)
    nc = get_program(n0)
    res = run_bass_kernel_spmd(
        nc, in_maps, core_ids=list(range(NCORES)), trace=False
    )
    return assemble_output(res.results, perm)


def _make_runner(nc, donate=True):
    """Build a jitted 8-core runner (mirrors bass2jax.run_bass_via_pjrt).
    With donate=False, inputs/zero-outs stay device-resident across calls,
    so repeated calls re-execute the NEFF without re-uploading data."""
    import jax
    import concourse.mybir as mybir
    from jax.sharding import Mesh, NamedSharding, PartitionSpec
    from jax.experimental.shard_map import shard_map
    from concourse.bass2jax import (
        _bass_exec_p,
        install_neuronx_cc_hook,
        partition_id_tensor,
    )

    install_neuronx_cc_hook()

    partition_name = (
        nc.partition_id_tensor.name if nc.partition_id_tensor else None
    )
    in_names, out_names, out_avals, zero_outs = [], [], [], []
    for alloc in nc.m.functions[0].allocations:
        if not isinstance(alloc, mybir.MemoryLocationSet):
            continue
        name = alloc.memorylocations[0].name
        if alloc.kind == "ExternalInput":
            if name != partition_name:
                in_names.append(name)
        elif alloc.kind == "ExternalOutput":
            out_names.append(name)
            shape = tuple(alloc.tensor_shape)
            dtype = mybir.dt.np(alloc.dtype)
            out_avals.append(jax.core.ShapedArray(shape, dtype))
            zero_outs.append(np.zeros(shape, dtype))
    n_params = len(in_names)
    n_outs = len(out_avals)
    all_names = list(in_names) + list(out_names)
    if partition_name is not None:
        all_names.append(partition_name)
    all_names = tuple(all_names)

    def _body(*args):
        operands = list(args)
        if partition_name is not None:
            operands.append(partition_id_tensor())
        outs = _bass_exec_p.bind(
            *operands,
            out_avals=tuple(out_avals),
            in_names=all_names,
            out_names=tuple(out_names),
            lowering_input_output_aliases=(),
            sim_require_finite=True,
            sim_require_nnan=True,
            nc=nc,
        )
        return tuple(outs)

    devices = jax.devices()[:NCORES]
    mesh = Mesh(np.asarray(devices), ("core",))
    in_specs = (PartitionSpec("core"),) * (n_params + n_outs)
    out_specs = (PartitionSpec("core"),) * n_outs
    sharded = jax.jit(
        shard_map(
            _body, mesh=mesh, in_specs=in_specs, out_specs=out_specs,
            check_rep=False,
        ),
        donate_argnums=(
            tuple(range(n_params, n_params + n_outs)) if donate else ()
        ),
        keep_unused=True,
    )

    sharding = NamedSharding(mesh, PartitionSpec("core"))

    def put(in_maps):
        import jax
        concat_in = [
            np.concatenate([in_maps[c][name] for c in range(NCORES)], axis=0)
            for name in in_names
        ]
        concat_zeros = [
            np.zeros((NCORES * z.shape[0], *z.shape[1:]), z.dtype)
            for z in zero_outs
        ]
        return [jax.device_put(a, sharding) for a in concat_in + concat_zeros]

    def unpack(out_arrs):
        return [
            {
                name: np.asarray(out_arrs[i]).reshape(
                    NCORES, *out_avals[i].shape
                )[c]
                for i, name in enumerate(out_names)
            }
            for c in range(NCORES)
        ]

    return sharded, put, unpack


def bench(x, W, A_buffer, B_buffer, weight_indices, iters=24):
    """Returns (output, per_exec_ns, info). Fires `iters` async executions
    with device-resident inputs and blocks at the end, so per-call dispatch
    overlaps execution; the amortized delta approximates HW exec time."""
    import time

    import jax

    perm, wis, n0 = _prep(weight_indices)
    in_maps = make_in_maps(x, W, A_buffer, B_buffer, weight_indices,
                           perm=perm, wis=wis)
    nc = get_program(n0)
    sharded, put, unpack = _make_runner(nc, donate=False)
    dev_args = put(in_maps)

    outs = jax.block_until_ready(sharded(*dev_args))  # compile + warm-up
    results = unpack(outs)

    def burst(k):
        t0 = time.monotonic()
        rs = [sharded(*dev_args) for _ in range(k)]
        jax.block_until_ready(rs)
        return time.monotonic() - t0

    # The tunnel adds a large, noisy fixed dispatch cost per burst
    # (~75ms +/- tens of ms).  Use wide bursts and min-statistics so
    # the per-exec marginal isn't swamped: noise is one-sided
    # (contention only adds time), so min over repeats converges on
    # the true pipeline time.
    iters = max(iters, 64)
    burst(4)  # extra warm-up
    t_small = min(burst(4) for _ in range(5))
    t_big = min(burst(4 + iters) for _ in range(5))
    per_exec_ns = (t_big - t_small) / iters * 1e9
    info = {
        "t_small_s": t_small,
        "t_big_s": t_big,
        "iters": iters,
        "per_exec_ns": per_exec_ns,
    }
    return assemble_output(results, perm), per_exec_ns, info


# revision 12
# speedup vs baseline: 4.7823x; 2.2481x over previous
# Trainium2 Bass kernel for MergedColumnParallelLinearWithTopping
# (base column-parallel GEMM + per-token LoRA "topping", Punica-style).
#
# Math per core c (of 8, column-parallel over the 2*BDIM output dim):
#   out_c = x @ Wc.T + ((x @ Ac) * Mc) @ Bc
# with tokens SORTED by adapter id on the host (the permutation folds
# into the host-side pack and un-pack, costing nothing on device).
# Sorting makes the LoRA block-structured: adapters 0-7 ("group 0",
# 128 lora rows) own a contiguous token range, adapters 8-15 ("group
# 1") own the rest.  Per 512-token slice at most one group boundary
# crosses, so
#   - the x@A matmuls for group g only run over group g's own tokens
#     (halves the dense-mask x@A cost), and
#   - each xa@B matmul needs a single K=128 chunk instead of K=256
#     (halves the dense-mask B cost).
# The one-hot mask M (applied on DVE, off the critical PE path) still
# zeroes the 7 sibling adapters within the group.  All matmuls run as
# float32r (full PE rate for moving free dim >= 256).
#
# Self-contained: hardcodes shapes, builds the Bass program (shaped by
# the group boundary n0 derived from weight_indices at call time),
# shards inputs, runs on cores 0-7 via run_bass_kernel_spmd,
# reassembles the full output.

import numpy as np

# Problem shapes (hardcoded per spec)
T, D = 2048, 2048
L, R = 16, 16
BDIM = 5632
NCORES = 8
CPC = 2 * BDIM // NCORES  # 1408 output cols per core
P = 128
KO = D // P               # 16 contraction chunks
TS = 512                  # token-slice (matmul moving free dim)
NT = T // TS              # 4
MCH = CPC // P            # 11 output-column chunks per core
LR = L * R                # 256 (one half's lora rows)
LRO = LR // P             # 2 adapter groups of 8 (128 lora rows each)
GSZ = L // LRO            # 8 adapters per group

_PROGRAM_CACHE = {}


def _prep(weight_indices):
    """Sort tokens by adapter; n0 = #tokens in adapter group 0."""
    wi = np.asarray(weight_indices).astype(np.int64).ravel()
    perm = np.argsort(wi, kind="stable")
    wis = wi[perm]
    n0 = int(np.sum(wis < GSZ))
    return perm, wis, n0


def _slice_ranges(n0):
    """Per token-slice t: list of (group, j0, j1) sub-ranges (within-
    slice columns [j0, j1) owned by that adapter group).  fp32r
    matmuls require an even moving free dim, so the group boundary is
    rounded outward to even positions: the <=2 overlap tokens are
    computed by both groups, and the wrong group's contribution is
    exactly zero because the one-hot mask zeroes its xam rows."""
    e0 = ((n0 + 1) // 2) * 2  # group-0 end, rounded up to even
    b1 = (n0 // 2) * 2        # group-1 begin, rounded down to even
    out = []
    for t in range(NT):
        lo, hi = t * TS, (t + 1) * TS
        rs = []
        if lo < e0:
            rs.append((0, 0, min(hi, e0) - lo))
        if hi > b1:
            rs.append((1, max(lo, b1) - lo, TS))
        out.append(rs)
    return out


def _build_program(n0, rep=1):
    """Build the Bass program.  rep>1 re-issues the full body (DMAs +
    compute) rep times into one NEFF, reusing the same SBUF tiles; WAR
    deps serialize the reps, mimicking back-to-back executions.  Used
    for measurement: one dispatch runs rep device executions, so the
    timing difference between rep and 1 isolates on-device time."""
    import concourse.bacc as bacc
    import concourse.tile as tile
    from concourse import mybir

    f32 = mybir.dt.float32
    f32r = mybir.dt.float32r
    bf16 = mybir.dt.bfloat16

    ranges = _slice_ranges(n0)

    nc = bacc.Bacc("TRN2", target_bir_lowering=False, debug=False)

    # All inputs arrive pre-packed on the host into SBUF layout, so every
    # DMA reads/writes long contiguous per-partition runs.
    xt_r = nc.dram_tensor("xt", [NT, P, KO, TS], f32r, kind="ExternalInput").ap()
    wt_r = nc.dram_tensor("wt", [MCH, P, KO, P], f32r, kind="ExternalInput").ap()
    ac_r = nc.dram_tensor("ac", [P, KO, LR], f32r, kind="ExternalInput").ap()
    bc_r = nc.dram_tensor("bc", [P, LRO, CPC], f32r, kind="ExternalInput").ap()
    mt_r = nc.dram_tensor("mt", [NT, P, LRO, TS], bf16, kind="ExternalInput").ap()
    out_r = nc.dram_tensor("out", [MCH, P, NT, TS], f32, kind="ExternalOutput").ap()

    with tile.TileContext(nc) as tc:
        with (
            tc.tile_pool(name="xres", bufs=NT) as xpool,
            tc.tile_pool(name="wpool", bufs=2) as wpool,
            tc.tile_pool(name="consts", bufs=1) as cpool,
            tc.tile_pool(name="mtp", bufs=NT) as mtpool,
            tc.tile_pool(name="outp", bufs=3) as outpool,
            tc.tile_pool(name="psout", bufs=4, space="PSUM") as psout,
            tc.tile_pool(name="psxa", bufs=2, space="PSUM") as psxa,
        ):
            # Split DMAs that feed matmul operands into k-groups: the fused
            # fp32r matmul's LDWEIGHTS has very few semaphore-wait slots, so
            # each matmul must depend on at most one small DMA.
            KG = 4  # k-chunks per sub-DMA

            # SBUF residents, allocated once (re-filled per rep)
            a_sb = cpool.tile([P, KO, LR], f32r, name="a_sb")
            b_sb = cpool.tile([P, LRO, CPC], f32r, name="b_sb")
            # masked x@A activation, filled per (group, token sub-range)
            xam = cpool.tile([P, LRO, T], f32r, name="xam")
            x_sb = [
                xpool.tile([P, KO, TS], f32r, name=f"x{t}", tag="x")
                for t in range(NT)
            ]

            for r in range(rep):
                for kg in range(0, KO, KG):
                    nc.sync.dma_start(
                        a_sb[:, kg:kg + KG, :], ac_r[:, kg:kg + KG, :]
                    )
                for o in range(LRO):
                    nc.sync.dma_start(b_sb[:, o, :], bc_r[:, o, :])

                # x fully resident, loaded as NT independent slices so deps
                # are per-slice (and per k-group within a slice)
                for t in range(NT):
                    for kg in range(0, KO, KG):
                        nc.sync.dma_start(
                            x_sb[t][:, kg:kg + KG, :],
                            xt_r[t, :, kg:kg + KG, :],
                        )

                def w_load(m, r=r):
                    # single DMA per chunk: one queue, so slot-reuse WAW is
                    # one semaphore and the guard absorbs the single wait
                    wtile = wpool.tile([P, KO, P], f32r, name=f"w{r}_{m}",
                                       tag="w")
                    nc.sync.dma_start(wtile[:], wt_r[m])
                    return wtile

                w_tiles = {0: w_load(0)}

                def base_group(m, wtile, t, r=r):
                    # one [128, TS] output tile: 16 base matmuls + the lora
                    # B matmuls for the groups present in this slice, all
                    # accumulating in the same PSUM bank.  The last base
                    # matmul closes the group over the full free range.
                    ps = psout.tile([P, TS], f32, name=f"ps{r}_{m}_{t}",
                                    tag="ps")
                    for k in range(KO - 1):
                        nc.tensor.matmul(
                            ps[:],
                            lhsT=wtile[:, k, :],
                            rhs=x_sb[t][:, k, :],
                            start=(k == 0),
                            stop=False,
                        )
                    for (g, j0, j1) in ranges[t]:
                        nc.tensor.matmul(
                            ps[:, j0:j1],
                            lhsT=b_sb[:, g, m * P:(m + 1) * P],
                            rhs=xam[:, g, t * TS + j0:t * TS + j1],
                            start=False,
                            stop=False,
                        )
                    k = KO - 1
                    nc.tensor.matmul(
                        ps[:],
                        lhsT=wtile[:, k, :],
                        rhs=x_sb[t][:, k, :],
                        start=False,
                        stop=True,
                    )
                    o = outpool.tile([P, TS], f32, name=f"o{r}_{m}_{t}",
                                     tag="o")
                    nc.any.tensor_copy(out=o[:], in_=ps[:])
                    nc.sync.dma_start(out_r[m, :, t, :], o[:])

                # Pass 1 over token-slices: compute masked XA for the
                # groups present in each slice, then the first W chunks
                for t in range(NT):
                    mt_sb = mtpool.tile([P, LRO, TS], bf16,
                                        name=f"mt{r}_{t}", tag="mt")
                    for (g, j0, j1) in ranges[t]:
                        nc.sync.dma_start(
                            mt_sb[:, g, j0:j1], mt_r[t, :, g, j0:j1]
                        )
                    for (g, j0, j1) in ranges[t]:
                        # full-bank PSUM tile (bank-aligned) sliced to the
                        # sub-range; odd-sized PSUM allocations can land at
                        # non-bank-aligned offsets the fp32r matmul rejects
                        pxa = psxa.tile([P, TS], f32, name=f"pxa{r}_{t}_{g}",
                                        tag="pxa")
                        for k in range(KO):
                            nc.tensor.matmul(
                                pxa[:, 0:j1 - j0],
                                lhsT=a_sb[:, k, g * P:(g + 1) * P],
                                rhs=x_sb[t][:, k, j0:j1],
                                start=(k == 0),
                                stop=(k == KO - 1),
                            )
                        nc.vector.tensor_tensor(
                            xam[:, g, t * TS + j0:t * TS + j1],
                            pxa[:, 0:j1 - j0],
                            mt_sb[:, g, j0:j1],
                            mybir.AluOpType.mult,
                        )
                    if t == 0:
                        w_tiles[1] = w_load(1)
                        w_tiles[2] = w_load(2)
                    base_group(0, w_tiles[0], t)
                    base_group(1, w_tiles[1], t)

                # Remaining W chunks, x stays resident
                for m in range(2, MCH):
                    if m + 1 < MCH and (m + 1) not in w_tiles:
                        w_tiles[m + 1] = w_load(m + 1)
                    for t in range(NT):
                        base_group(m, w_tiles[m], t)

    nc.compile()
    return nc


def get_program(n0, rep=1):
    key = (n0, rep)
    if key not in _PROGRAM_CACHE:
        _PROGRAM_CACHE[key] = _build_program(n0, rep)
    return _PROGRAM_CACHE[key]


def make_in_maps(x, W, A_buffer, B_buffer, weight_indices, perm=None,
                 wis=None):
    x = np.ascontiguousarray(np.asarray(x, dtype=np.float32))
    W = np.asarray(W, dtype=np.float32)
    A = np.asarray(A_buffer, dtype=np.float32)
    B = np.asarray(B_buffer, dtype=np.float32)
    if perm is None:
        perm, wis, _ = _prep(weight_indices)

    xs_srt = x[perm]
    # pack to SBUF layout [.., P(partition), .., contiguous free dims]
    xt = np.ascontiguousarray(
        xs_srt.T.reshape(KO, P, NT, TS).transpose(2, 1, 0, 3)
    )  # [NT, P, KO, TS]
    onehot = (wis[None, :] == np.arange(L, dtype=wis.dtype)[:, None])
    import ml_dtypes
    mt = np.ascontiguousarray(
        np.repeat(onehot, R, axis=0)
        .reshape(LRO, P, NT, TS)
        .transpose(2, 1, 0, 3)
    ).astype(ml_dtypes.bfloat16)  # [NT, P, LRO, TS]

    in_maps = []
    for c in range(NCORES):
        h = c // 4
        lo = h * BDIM + (c % 4) * CPC
        gcols = slice(lo, lo + CPC)
        wt_c = np.ascontiguousarray(
            W[gcols, :].T.reshape(KO, P, MCH, P).transpose(2, 1, 0, 3)
        )  # [MCH, P, KO, P]
        ac_c = np.ascontiguousarray(
            A[:, :, h * R:(h + 1) * R]
            .transpose(1, 0, 2).reshape(KO, P, LR).transpose(1, 0, 2)
        )  # [P, KO, LR]
        bc_c = np.ascontiguousarray(
            B[:, :, gcols].reshape(LRO, P, CPC).transpose(1, 0, 2)
        )  # [P, LRO, CPC]
        in_maps.append({"xt": xt, "wt": wt_c, "ac": ac_c, "bc": bc_c, "mt": mt})
    return in_maps


def assemble_output(results, perm):
    out = np.empty((T, 2 * BDIM), dtype=np.float32)
    for c in range(NCORES):
        h = c // 4
        lo = h * BDIM + (c % 4) * CPC
        # [MCH, P, NT, TS] -> [sorted tok, col]
        piece = results[c]["out"].transpose(2, 3, 0, 1).reshape(T, CPC)
        out[perm, lo:lo + CPC] = piece
    return out


def kernel(x, W, A_buffer, B_buffer, weight_indices):
    from concourse.bass_utils import run_bass_kernel_spmd

    perm, wis, n0 = _prep(weight_indices)
    in_maps = make_in_maps(x, W, A_buffer, B_buffer, weight_indices,
                           perm=perm, wis=wis)
    nc = get_program(n0)
    res = run_bass_kernel_spmd(
        nc, in_maps, core_ids=list(range(NCORES)), trace=False
    )
    return assemble_output(res.results, perm)


def _make_runner(nc, donate=True):
    """Build a jitted 8-core runner (mirrors bass2jax.run_bass_via_pjrt).
    With donate=False, inputs/zero-outs stay device-resident across calls,
    so repeated calls re-execute the NEFF without re-uploading data."""
    import jax
    import concourse.mybir as mybir
    from jax.sharding import Mesh, NamedSharding, PartitionSpec
    from jax.experimental.shard_map import shard_map
    from concourse.bass2jax import (
        _bass_exec_p,
        install_neuronx_cc_hook,
        partition_id_tensor,
    )

    install_neuronx_cc_hook()

    partition_name = (
        nc.partition_id_tensor.name if nc.partition_id_tensor else None
    )
    in_names, out_names, out_avals, zero_outs = [], [], [], []
    for alloc in nc.m.functions[0].allocations:
        if not isinstance(alloc, mybir.MemoryLocationSet):
            continue
        name = alloc.memorylocations[0].name
        if alloc.kind == "ExternalInput":
            if name != partition_name:
                in_names.append(name)
        elif alloc.kind == "ExternalOutput":
            out_names.append(name)
            shape = tuple(alloc.tensor_shape)
            dtype = mybir.dt.np(alloc.dtype)
            out_avals.append(jax.core.ShapedArray(shape, dtype))
            zero_outs.append(np.zeros(shape, dtype))
    n_params = len(in_names)
    n_outs = len(out_avals)
    all_names = list(in_names) + list(out_names)
    if partition_name is not None:
        all_names.append(partition_name)
    all_names = tuple(all_names)

    def _body(*args):
        operands = list(args)
        if partition_name is not None:
            operands.append(partition_id_tensor())
        outs = _bass_exec_p.bind(
            *operands,
            out_avals=tuple(out_avals),
            in_names=all_names,
            out_names=tuple(out_names),
            lowering_input_output_aliases=(),
            sim_require_finite=True,
            sim_require_nnan=True,
            nc=nc,
        )
        return tuple(outs)

    devices = jax.devices()[:NCORES]
    mesh = Mesh(np.asarray(devices), ("core",))
    in_specs = (PartitionSpec("core"),) * (n_params + n_outs)
    out_specs = (PartitionSpec("core"),) * n_outs
    sharded = jax.jit(
        shard_map(
            _body, mesh=mesh, in_specs=in_specs, out_specs=out_specs,
            check_rep=False,
        ),
        donate_argnums=(
            tuple(range(n_params, n_params + n_outs)) if donate else ()
        ),
        keep_unused=True,
    )

    sharding = NamedSharding(mesh, PartitionSpec("core"))

    def put(in_maps):
        import jax
        concat_in = [
            np.concatenate([in_maps[c][name] for c in range(NCORES)], axis=0)
            for name in in_names
        ]
        concat_zeros = [
            np.zeros((NCORES * z.shape[0], *z.shape[1:]), z.dtype)
            for z in zero_outs
        ]
        return [jax.device_put(a, sharding) for a in concat_in + concat_zeros]

    def unpack(out_arrs):
        return [
            {
                name: np.asarray(out_arrs[i]).reshape(
                    NCORES, *out_avals[i].shape
                )[c]
                for i, name in enumerate(out_names)
            }
            for c in range(NCORES)
        ]

    return sharded, put, unpack


def bench(x, W, A_buffer, B_buffer, weight_indices, iters=24):
    """Returns (output, per_exec_ns, info). Fires `iters` async executions
    with device-resident inputs and blocks at the end, so per-call dispatch
    overlaps execution; the amortized delta approximates HW exec time."""
    import time

    import jax

    perm, wis, n0 = _prep(weight_indices)
    in_maps = make_in_maps(x, W, A_buffer, B_buffer, weight_indices,
                           perm=perm, wis=wis)
    # Two program variants: the real one, and one whose NEFF re-runs the
    # full body REP times back-to-back on device.  The per-call time
    # difference divided by REP-1 isolates on-device time: per-dispatch
    # tunnel overhead (0.5-1.5ms and highly variable here) cancels.
    REP = 8
    nc1 = get_program(n0, rep=1)
    ncR = get_program(n0, rep=REP)
    sharded1, put, unpack = _make_runner(nc1, donate=False)
    shardedR, putR, _ = _make_runner(ncR, donate=False)
    dev_args = put(in_maps)
    dev_argsR = putR(in_maps)

    outs = jax.block_until_ready(sharded1(*dev_args))  # compile + warm-up
    results = unpack(outs)
    jax.block_until_ready(shardedR(*dev_argsR))

    def burst(sh, args, k):
        t0 = time.monotonic()
        rs = [sh(*args) for _ in range(k)]
        jax.block_until_ready(rs)
        return time.monotonic() - t0

    NCALLS = 16
    burst(sharded1, dev_args, 4)  # extra warm-up
    t1s, tRs = [], []
    for _ in range(6):
        t1s.append(burst(sharded1, dev_args, NCALLS))
        tRs.append(burst(shardedR, dev_argsR, NCALLS))
    t1, tR = min(t1s), min(tRs)
    per_exec_ns = (tR - t1) / (NCALLS * (REP - 1)) * 1e9
    info = {
        "t_rep1_s": t1,
        "t_repR_s": tR,
        "rep": REP,
        "ncalls": NCALLS,
        "iters": iters,
        "per_exec_ns": per_exec_ns,
        "overhead_cancelled": True,
    }
    return assemble_output(results, perm), per_exec_ns, info


# revision 18
# speedup vs baseline: 8.0439x; 1.6820x over previous
# Trainium2 Bass kernel for MergedColumnParallelLinearWithTopping
# (base column-parallel GEMM + per-token LoRA "topping", Punica-style).
#
# Math per core c (of 8, column-parallel over the 2*BDIM output dim):
#   out_c = x @ Wc.T + ((x @ Ac) * Mc) @ Bc
# with tokens SORTED by adapter id on the host (the permutation folds
# into the host-side pack and un-pack, costing nothing on device).
# Sorting makes the LoRA block-structured: adapters 0-7 ("group 0",
# 128 lora rows) own a contiguous token range, adapters 8-15 ("group
# 1") own the rest.  Per 512-token slice at most one group boundary
# crosses, so
#   - the x@A matmuls for group g only run over group g's own tokens
#     (halves the dense-mask x@A cost), and
#   - each xa@B matmul needs a single K=128 chunk instead of K=256
#     (halves the dense-mask B cost).
# The one-hot mask M (applied on DVE, off the critical PE path) still
# zeroes the 7 sibling adapters within the group.  x/W/A ship and
# stream as bf16 (halves the x/W DMA; error ~2.4e-3 of output scale,
# well inside the 2e-2 gate); the B matmuls run as float32r.
#
# Self-contained: hardcodes shapes, builds the Bass program (shaped by
# the group boundary n0 derived from weight_indices at call time),
# shards inputs, runs on cores 0-7 via run_bass_kernel_spmd,
# reassembles the full output.

import numpy as np

# Problem shapes (hardcoded per spec)
T, D = 2048, 2048
L, R = 16, 16
BDIM = 5632
NCORES = 8
CPC = 2 * BDIM // NCORES  # 1408 output cols per core
P = 128
KO = D // P               # 16 contraction chunks
TS = 512                  # token-slice (matmul moving free dim)
NT = T // TS              # 4
MCH = CPC // P            # 11 output-column chunks per core
LR = L * R                # 256 (one half's lora rows)
LRO = LR // P             # 2 adapter groups of 8 (128 lora rows each)
GSZ = L // LRO            # 8 adapters per group

_PROGRAM_CACHE = {}


def _prep(weight_indices):
    """Sort tokens by adapter; n0 = #tokens in adapter group 0."""
    wi = np.asarray(weight_indices).astype(np.int64).ravel()
    perm = np.argsort(wi, kind="stable")
    wis = wi[perm]
    n0 = int(np.sum(wis < GSZ))
    return perm, wis, n0


def _slice_ranges(n0):
    """Per token-slice t: list of (group, j0, j1) sub-ranges (within-
    slice columns [j0, j1) owned by that adapter group).  fp32r
    matmuls require an even moving free dim, so the group boundary is
    rounded outward to even positions: the <=2 overlap tokens are
    computed by both groups, and the wrong group's contribution is
    exactly zero because the one-hot mask zeroes its xam rows."""
    e0 = ((n0 + 1) // 2) * 2  # group-0 end, rounded up to even
    b1 = (n0 // 2) * 2        # group-1 begin, rounded down to even
    out = []
    for t in range(NT):
        lo, hi = t * TS, (t + 1) * TS
        rs = []
        if lo < e0:
            rs.append((0, 0, min(hi, e0) - lo))
        if hi > b1:
            rs.append((1, max(lo, b1) - lo, TS))
        out.append(rs)
    return out


def _build_program(n0, rep=1, variant="sorted", x_resident=False):
    """Build the Bass program.  rep>1 re-issues the full body (DMAs +
    compute) rep times into one NEFF, reusing the same SBUF tiles; WAR
    deps serialize the reps, mimicking back-to-back executions.  Used
    for measurement: one dispatch runs rep device executions, so the
    timing difference between rep and 1 isolates on-device time.

    variant: "sorted" (real kernel), "dense" (baseline-equivalent dense
    mask: both groups over all slices), "base" (base GEMM only, no
    LoRA) — the latter two only for attribution measurements.
    x_resident=True loads x once outside the rep loop (attribution of
    the per-exec x-DMA serialization head)."""
    import concourse.bacc as bacc
    import concourse.tile as tile
    from concourse import mybir

    f32 = mybir.dt.float32
    f32r = mybir.dt.float32r
    bf16 = mybir.dt.bfloat16

    if variant == "dense":
        ranges = [[(0, 0, TS), (1, 0, TS)] for _ in range(NT)]
    elif variant == "base":
        ranges = [[] for _ in range(NT)]
    else:
        ranges = _slice_ranges(n0)

    nc = bacc.Bacc("TRN2", target_bir_lowering=False, debug=False)

    # All inputs arrive pre-packed on the host into SBUF layout, so every
    # DMA reads/writes long contiguous per-partition runs.
    xt_r = nc.dram_tensor("xt", [NT, P, KO, TS], bf16, kind="ExternalInput").ap()
    wt_r = nc.dram_tensor("wt", [MCH, P, KO, P], bf16, kind="ExternalInput").ap()
    ac_r = nc.dram_tensor("ac", [P, KO, LR], bf16, kind="ExternalInput").ap()
    bc_r = nc.dram_tensor("bc", [P, LRO, CPC], f32r, kind="ExternalInput").ap()
    mt_r = nc.dram_tensor("mt", [NT, P, LRO, TS], bf16, kind="ExternalInput").ap()
    out_r = nc.dram_tensor("out", [MCH, P, NT, TS], f32, kind="ExternalOutput").ap()

    with tile.TileContext(nc) as tc:
        with (
            tc.tile_pool(name="xres", bufs=NT) as xpool,
            tc.tile_pool(name="wpool", bufs=2) as wpool,
            tc.tile_pool(name="consts", bufs=1) as cpool,
            tc.tile_pool(name="mtp", bufs=NT) as mtpool,
            tc.tile_pool(name="outp", bufs=3) as outpool,
            tc.tile_pool(name="psout", bufs=4, space="PSUM") as psout,
            tc.tile_pool(name="psxa", bufs=2, space="PSUM") as psxa,
        ):
            # Split DMAs that feed matmul operands into k-groups: the fused
            # fp32r matmul's LDWEIGHTS has very few semaphore-wait slots, so
            # each matmul must depend on at most one small DMA.
            KG = 4  # k-chunks per sub-DMA

            # SBUF residents, allocated once (re-filled per rep)
            a_sb = cpool.tile([P, KO, LR], bf16, name="a_sb")
            b_sb = cpool.tile([P, LRO, CPC], f32r, name="b_sb")
            # masked x@A activation, filled per (group, token sub-range)
            xam = cpool.tile([P, LRO, T], f32r, name="xam")
            x_sb = [
                xpool.tile([P, KO, TS], bf16, name=f"x{t}", tag="x")
                for t in range(NT)
            ]

            for r in range(rep):
                for kg in range(0, KO, KG):
                    nc.sync.dma_start(
                        a_sb[:, kg:kg + KG, :], ac_r[:, kg:kg + KG, :]
                    )
                for o in range(LRO):
                    nc.sync.dma_start(b_sb[:, o, :], bc_r[:, o, :])

                # x fully resident, loaded as NT independent slices so deps
                # are per-slice (and per k-group within a slice)
                if r == 0 or not x_resident:
                    for t in range(NT):
                        for kg in range(0, KO, KG):
                            nc.sync.dma_start(
                                x_sb[t][:, kg:kg + KG, :],
                                xt_r[t, :, kg:kg + KG, :],
                            )

                def w_load(m, r=r):
                    # single DMA per chunk: one queue, so slot-reuse WAW is
                    # one semaphore and the guard absorbs the single wait
                    wtile = wpool.tile([P, KO, P], bf16, name=f"w{r}_{m}",
                                       tag="w")
                    nc.sync.dma_start(wtile[:], wt_r[m])
                    return wtile

                w_tiles = {0: w_load(0)}

                def base_group(m, wtile, t, r=r):
                    # one [128, TS] output tile: 16 base matmuls + the lora
                    # B matmuls for the groups present in this slice, all
                    # accumulating in the same PSUM bank.  The last base
                    # matmul closes the group over the full free range.
                    ps = psout.tile([P, TS], f32, name=f"ps{r}_{m}_{t}",
                                    tag="ps")
                    for k in range(KO - 1):
                        nc.tensor.matmul(
                            ps[:],
                            lhsT=wtile[:, k, :],
                            rhs=x_sb[t][:, k, :],
                            start=(k == 0),
                            stop=False,
                        )
                    for (g, j0, j1) in ranges[t]:
                        nc.tensor.matmul(
                            ps[:, j0:j1],
                            lhsT=b_sb[:, g, m * P:(m + 1) * P],
                            rhs=xam[:, g, t * TS + j0:t * TS + j1],
                            start=False,
                            stop=False,
                        )
                    k = KO - 1
                    nc.tensor.matmul(
                        ps[:],
                        lhsT=wtile[:, k, :],
                        rhs=x_sb[t][:, k, :],
                        start=False,
                        stop=True,
                    )
                    o = outpool.tile([P, TS], f32, name=f"o{r}_{m}_{t}",
                                     tag="o")
                    nc.any.tensor_copy(out=o[:], in_=ps[:])
                    nc.sync.dma_start(out_r[m, :, t, :], o[:])

                # Pass 1 over token-slices: compute masked XA for the
                # groups present in each slice, then the first W chunks
                for t in range(NT):
                    mt_sb = mtpool.tile([P, LRO, TS], bf16,
                                        name=f"mt{r}_{t}", tag="mt")
                    for (g, j0, j1) in ranges[t]:
                        nc.sync.dma_start(
                            mt_sb[:, g, j0:j1], mt_r[t, :, g, j0:j1]
                        )
                    for (g, j0, j1) in ranges[t]:
                        # full-bank PSUM tile (bank-aligned) sliced to the
                        # sub-range; odd-sized PSUM allocations can land at
                        # non-bank-aligned offsets the fp32r matmul rejects
                        pxa = psxa.tile([P, TS], f32, name=f"pxa{r}_{t}_{g}",
                                        tag="pxa")
                        for k in range(KO):
                            nc.tensor.matmul(
                                pxa[:, 0:j1 - j0],
                                lhsT=a_sb[:, k, g * P:(g + 1) * P],
                                rhs=x_sb[t][:, k, j0:j1],
                                start=(k == 0),
                                stop=(k == KO - 1),
                            )
                        nc.vector.tensor_tensor(
                            xam[:, g, t * TS + j0:t * TS + j1],
                            pxa[:, 0:j1 - j0],
                            mt_sb[:, g, j0:j1],
                            mybir.AluOpType.mult,
                        )
                    if t == 0:
                        w_tiles[1] = w_load(1)
                        w_tiles[2] = w_load(2)
                    base_group(0, w_tiles[0], t)
                    base_group(1, w_tiles[1], t)

                # Remaining W chunks, x stays resident
                for m in range(2, MCH):
                    if m + 1 < MCH and (m + 1) not in w_tiles:
                        w_tiles[m + 1] = w_load(m + 1)
                    for t in range(NT):
                        base_group(m, w_tiles[m], t)

    nc.compile()
    return nc


def get_program(n0, rep=1, variant="sorted", x_resident=False):
    key = (n0, rep, variant, x_resident)
    if key not in _PROGRAM_CACHE:
        _PROGRAM_CACHE[key] = _build_program(n0, rep, variant, x_resident)
    return _PROGRAM_CACHE[key]


def make_in_maps(x, W, A_buffer, B_buffer, weight_indices, perm=None,
                 wis=None):
    import ml_dtypes
    bf16 = ml_dtypes.bfloat16
    x = np.ascontiguousarray(np.asarray(x, dtype=np.float32))
    W = np.asarray(W, dtype=np.float32)
    A = np.asarray(A_buffer, dtype=np.float32)
    B = np.asarray(B_buffer, dtype=np.float32)
    if perm is None:
        perm, wis, _ = _prep(weight_indices)

    xs_srt = x[perm]
    # pack to SBUF layout [.., P(partition), .., contiguous free dims];
    # x/W/A ship as bf16 (the PE upconverts to fp22 — error ~2.4e-3 of
    # output scale, well under the 2e-2 gate), B stays fp32r.
    xt = np.ascontiguousarray(
        xs_srt.T.reshape(KO, P, NT, TS).transpose(2, 1, 0, 3)
    ).astype(bf16)  # [NT, P, KO, TS]
    onehot = (wis[None, :] == np.arange(L, dtype=wis.dtype)[:, None])
    mt = np.ascontiguousarray(
        np.repeat(onehot, R, axis=0)
        .reshape(LRO, P, NT, TS)
        .transpose(2, 1, 0, 3)
    ).astype(bf16)  # [NT, P, LRO, TS]

    in_maps = []
    for c in range(NCORES):
        h = c // 4
        lo = h * BDIM + (c % 4) * CPC
        gcols = slice(lo, lo + CPC)
        wt_c = np.ascontiguousarray(
            W[gcols, :].T.reshape(KO, P, MCH, P).transpose(2, 1, 0, 3)
        ).astype(bf16)  # [MCH, P, KO, P]
        ac_c = np.ascontiguousarray(
            A[:, :, h * R:(h + 1) * R]
            .transpose(1, 0, 2).reshape(KO, P, LR).transpose(1, 0, 2)
        ).astype(bf16)  # [P, KO, LR]
        bc_c = np.ascontiguousarray(
            B[:, :, gcols].reshape(LRO, P, CPC).transpose(1, 0, 2)
        )  # [P, LRO, CPC]
        in_maps.append({"xt": xt, "wt": wt_c, "ac": ac_c, "bc": bc_c, "mt": mt})
    return in_maps


def assemble_output(results, perm):
    out = np.empty((T, 2 * BDIM), dtype=np.float32)
    for c in range(NCORES):
        h = c // 4
        lo = h * BDIM + (c % 4) * CPC
        # [MCH, P, NT, TS] -> [sorted tok, col]
        piece = results[c]["out"].transpose(2, 3, 0, 1).reshape(T, CPC)
        out[perm, lo:lo + CPC] = piece
    return out


def kernel(x, W, A_buffer, B_buffer, weight_indices):
    from concourse.bass_utils import run_bass_kernel_spmd

    perm, wis, n0 = _prep(weight_indices)
    in_maps = make_in_maps(x, W, A_buffer, B_buffer, weight_indices,
                           perm=perm, wis=wis)
    nc = get_program(n0)
    res = run_bass_kernel_spmd(
        nc, in_maps, core_ids=list(range(NCORES)), trace=False
    )
    return assemble_output(res.results, perm)


def _make_runner(nc, donate=True):
    """Build a jitted 8-core runner (mirrors bass2jax.run_bass_via_pjrt).
    With donate=False, inputs/zero-outs stay device-resident across calls,
    so repeated calls re-execute the NEFF without re-uploading data."""
    import jax
    import concourse.mybir as mybir
    from jax.sharding import Mesh, NamedSharding, PartitionSpec
    from jax.experimental.shard_map import shard_map
    from concourse.bass2jax import (
        _bass_exec_p,
        install_neuronx_cc_hook,
        partition_id_tensor,
    )

    install_neuronx_cc_hook()

    partition_name = (
        nc.partition_id_tensor.name if nc.partition_id_tensor else None
    )
    in_names, out_names, out_avals, zero_outs = [], [], [], []
    for alloc in nc.m.functions[0].allocations:
        if not isinstance(alloc, mybir.MemoryLocationSet):
            continue
        name = alloc.memorylocations[0].name
        if alloc.kind == "ExternalInput":
            if name != partition_name:
                in_names.append(name)
        elif alloc.kind == "ExternalOutput":
            out_names.append(name)
            shape = tuple(alloc.tensor_shape)
            dtype = mybir.dt.np(alloc.dtype)
            out_avals.append(jax.core.ShapedArray(shape, dtype))
            zero_outs.append(np.zeros(shape, dtype))
    n_params = len(in_names)
    n_outs = len(out_avals)
    all_names = list(in_names) + list(out_names)
    if partition_name is not None:
        all_names.append(partition_name)
    all_names = tuple(all_names)

    def _body(*args):
        operands = list(args)
        if partition_name is not None:
            operands.append(partition_id_tensor())
        outs = _bass_exec_p.bind(
            *operands,
            out_avals=tuple(out_avals),
            in_names=all_names,
            out_names=tuple(out_names),
            lowering_input_output_aliases=(),
            sim_require_finite=True,
            sim_require_nnan=True,
            nc=nc,
        )
        return tuple(outs)

    devices = jax.devices()[:NCORES]
    mesh = Mesh(np.asarray(devices), ("core",))
    in_specs = (PartitionSpec("core"),) * (n_params + n_outs)
    out_specs = (PartitionSpec("core"),) * n_outs
    sharded = jax.jit(
        shard_map(
            _body, mesh=mesh, in_specs=in_specs, out_specs=out_specs,
            check_rep=False,
        ),
        donate_argnums=(
            tuple(range(n_params, n_params + n_outs)) if donate else ()
        ),
        keep_unused=True,
    )

    sharding = NamedSharding(mesh, PartitionSpec("core"))

    def put(in_maps):
        import jax
        concat_in = [
            np.concatenate([in_maps[c][name] for c in range(NCORES)], axis=0)
            for name in in_names
        ]
        concat_zeros = [
            np.zeros((NCORES * z.shape[0], *z.shape[1:]), z.dtype)
            for z in zero_outs
        ]
        return [jax.device_put(a, sharding) for a in concat_in + concat_zeros]

    def unpack(out_arrs):
        return [
            {
                name: np.asarray(out_arrs[i]).reshape(
                    NCORES, *out_avals[i].shape
                )[c]
                for i, name in enumerate(out_names)
            }
            for c in range(NCORES)
        ]

    return sharded, put, unpack


def bench(x, W, A_buffer, B_buffer, weight_indices, iters=24):
    """Returns (output, per_exec_ns, info). Fires `iters` async executions
    with device-resident inputs and blocks at the end, so per-call dispatch
    overlaps execution; the amortized delta approximates HW exec time."""
    import time

    import jax

    perm, wis, n0 = _prep(weight_indices)
    in_maps = make_in_maps(x, W, A_buffer, B_buffer, weight_indices,
                           perm=perm, wis=wis)
    # Two program variants: the real one, and one whose NEFF re-runs the
    # full body REP times back-to-back on device.  The per-call time
    # difference divided by REP-1 isolates on-device time: per-dispatch
    # tunnel overhead (0.5-1.5ms and highly variable here) cancels.
    REP = 8
    nc1 = get_program(n0, rep=1)
    ncR = get_program(n0, rep=REP)
    sharded1, put, unpack = _make_runner(nc1, donate=False)
    shardedR, putR, _ = _make_runner(ncR, donate=False)
    dev_args = put(in_maps)
    dev_argsR = putR(in_maps)

    outs = jax.block_until_ready(sharded1(*dev_args))  # compile + warm-up
    results = unpack(outs)
    jax.block_until_ready(shardedR(*dev_argsR))

    def burst(sh, args, k):
        t0 = time.monotonic()
        rs = [sh(*args) for _ in range(k)]
        jax.block_until_ready(rs)
        return time.monotonic() - t0

    NCALLS = 16
    burst(sharded1, dev_args, 4)  # extra warm-up
    t1s, tRs = [], []
    for _ in range(6):
        t1s.append(burst(sharded1, dev_args, NCALLS))
        tRs.append(burst(shardedR, dev_argsR, NCALLS))
    t1, tR = min(t1s), min(tRs)
    per_exec_ns = (tR - t1) / (NCALLS * (REP - 1)) * 1e9
    info = {
        "t_rep1_s": t1,
        "t_repR_s": tR,
        "rep": REP,
        "ncalls": NCALLS,
        "iters": iters,
        "per_exec_ns": per_exec_ns,
        "overhead_cancelled": True,
    }
    return assemble_output(results, perm), per_exec_ns, info
